# revision 1
# baseline (speedup 1.0000x reference)
"""GCN encoder (nn_GCNEncoder) Trainium2 Bass kernel — bf16 feature-major.

Math: with a fully-connected graph + self loops, gcn_norm gives the uniform
adjacency A = 1/N. Then A @ X broadcasts mean_n(X) to every node, so after
layer 1 the node features are constant within each graph and the whole GCN
collapses to a per-graph vector chain:

  locbar[b] = mean_n locs[b, n, :]                       (R^2)
  g0[b]     = locbar[b] @ W_init + b_init                (R^D)
  g1        = relu(g0 @ Ws[0] + bs[0]); g2 = relu(g1 @ Ws[1] + bs[1])
  g3        = g2 @ Ws[2] + bs[2]
  init_h[b, n, :]  = locs[b, n, :] @ W_init + b_init
  h_final[b, n, :] = init_h[b, n, :] + g3[b, :]

Outputs (h_final, init_h) are 2 x [2048, 100, 128]; the kernel is purely
store-bandwidth bound. Both outputs are stored as bf16 (measured rel err
3.9e-3 vs the 2e-2 gate) in FEATURE-MAJOR layout [D, T] per core: every
partition writes 6.4 KB contiguous DRAM lines, keeping the DMA engines at
full rate with 2-byte elements; the host transposes after the gather.
Store traffic: 13.1 MB/core (~37 us at the modeled ~330 GB/s) vs ~73 us
for f32 — the baseline this replaces ran ~80-95 us.

Per-core schedule (256 graphs = 25600 tokens, 8 chunks of 32 graphs):
 - moving strip [64, 3200] bf16 per chunk (3 rotating buffers, prefetched
   a chunk ahead on the SP ring): rows 0..7 = [xh yh xh yh xl yl 1 1] per
   token (locs split bf16 hi+lo), rows 8..31 zero pad (matmul/transpose
   outputs must start at psum partition 0/32/64), rows 32..63 = graph
   one-hot (sel), pad+sel loaded once per buffer.
 - stationaries for all 8 chunks in one persistent [64, 8*128] tile:
   rows 0..7 = [W0h W1h W0l W1l W0h W1h bh bl] (bf16 products exact in
   f32 PSUM; dropped xl*Wl term ~2^-18 keeps init_h f32-exact before the
   store rounding), rows 8..31 zeros, rows 32..63 = bf16 g3 rows written
   by per-chunk PE transposes straight out of the chain (no staging
   DMAs). The init output uses rows 0..7 only.
 - per chunk: 8 init matmuls then 8 final matmuls of 400 cols (PE ~21
   us/core total), psum pool 3 x [128, 1024] f32; 800-col PSUM->SBUF
   bf16 evacuations alternate DVE/ACT (GPSIMD cannot access PSUM);
   half-chunk stores issue as soon as their evacs land.
 - g3 chain: one F=256 pass in bf16 (wmean matmul + 3 layers), dedicated
   1-bank psum pools, locbar via DVE reduces + PE transposes. Emission
   order exploits per-engine program order: chunk 0/1 INIT phases are
   emitted before the chain's PE/ACT/DVE ops, so the store stream starts
   ~7 us in while the chain computes; rings: SP = consts/strips/stores,
   Pool(gpsimd) = statsall memset + sel loads (serial ~1 us/DMA SWDGE).
 - measured on trn2 (8 cores): ~54-56 us/iteration steady-state
   (rep-delta), predicted ~52 us single-shot; rel err 3.9e-3.
"""

import numpy as np
from contextlib import ExitStack

import concourse.bass as bass
import concourse.mybir as mybir
import concourse.tile as tile
from concourse.bass_utils import run_bass_kernel_spmd

F32 = mybir.dt.float32
BF16 = mybir.dt.bfloat16
AF = mybir.ActivationFunctionType

B, N, D, L = 2048, 100, 128, 3
NCORES = 8
BG = B // NCORES          # 256 graphs per core
T = BG * N                # 25600 tokens per core
CH = 8                    # chunks per core
GPC = BG // CH            # 32 graphs per chunk
TKC = GPC * N             # 3200 tokens per chunk
KB = 8                    # base K rows (locs hi/lo pairs + ones)
KS = 32                   # sel/g3 block base partition (0/32/64 rule)
KK = KS + GPC             # 64 contraction rows (8..31 are zero pad)
MT = 400                  # matmul moving-tile columns
RT = 2 * MT               # columns per psum tile (2 banks)
NR = TKC // RT            # 4 rounds per chunk per output
NSTRIP = 3                # rotating strip buffers

# evacuation engine per (chunk parity, round, output): 9 ACT / 7 DVE
# per two chunks, matching the 1.2 : 0.96 GHz engine rates.
EVAC_PAT = [["A", "V", "A", "V", "A", "V", "A", "A"],
            ["V", "A", "V", "A", "V", "A", "V", "A"]]


def _split_multiwaits(nc, max_waits=1):
    """The walrus build in this container rejects instructions carrying more
    than one sync-wait command. Split extras into single-wait NoOps inserted
    immediately before the instruction (same engine, so sequencer order
    preserves semantics exactly)."""
    cnt = 0
    for f in nc.m.functions:
        for b in f.blocks:
            il = b.instructions
            i = 0
            while i < len(il):
                ins = il[i]
                si = ins.sync_info
                if si is not None and si.on_wait and len(si.on_wait) > max_waits:
                    waits = list(si.on_wait)
                    for w in waits[:-max_waits]:
                        nop = mybir.InstNoOp(name=f"I-SWAIT-{cnt}", ins=[], outs=[])
                        cnt += 1
                        nop.engine = ins.engine
                        nop.sync_info = mybir.SyncInfo(on_wait=[w], on_update=[])
                        il.insert(i, nop)
                        i += 1
                    ins.sync_info = mybir.SyncInfo(
                        on_wait=waits[-max_waits:],
                        on_update=list(si.on_update or []))
                i += 1
    return cnt


def _build_program(split=True, reps=1):
    nc = bass.Bass("TRN2", target_bir_lowering=False, debug=False,
                   num_devices=NCORES)

    ins = {}
    for name, shape, dt in [
        ("master", [KB, T], BF16),         # coord rows + ones
        ("selpad", [KK - KB, TKC], BF16),  # 24 zero rows | 32 sel rows
        ("smallbf", [KB, 2 * D], BF16),    # cols 0:128 statc, 128:256 wmean (rows 0..1)
        ("fb32", [D, 4], F32),             # col 0 b_init, 1..3 bs.T
        ("wsall", [D, 4 * D], BF16),       # ident | Ws0 | Ws1 | Ws2
        ("locs2", [D, 4 * N], F32),        # graph p | graph p+128
    ]:
        ins[name] = nc.dram_tensor(name, shape, dt, kind="ExternalInput").ap()

    out_final = nc.dram_tensor("out_final", [D, T], BF16, kind="ExternalOutput").ap()
    out_init = nc.dram_tensor("out_init", [D, T], BF16, kind="ExternalOutput").ap()

    with tile.TileContext(nc) as tc, ExitStack() as ctx:
        const = ctx.enter_context(tc.tile_pool(name="const", bufs=1))

        # chain locs + chunk-0 strip on the SP ring first (critical
        # path); packed consts + sel on the Pool ring (a serial ~1 us/DMA
        # software-DGE path, ordered by first use) so the ACT/DVE
        # sequencers never stall behind DMA issues.
        lg = const.tile([D, 4 * N], F32, tag="lg")
        nc.sync.dma_start(lg[:], ins["locs2"][:])

        smallbf_sb = const.tile([KB, 2 * D], BF16, tag="smallbf")
        nc.sync.dma_start(smallbf_sb[:], ins["smallbf"][:])
        wsall_sb = const.tile([D, 4 * D], BF16, tag="wsall")
        nc.sync.dma_start(wsall_sb[:], ins["wsall"][:])
        fb32_sb = const.tile([D, 4], F32, tag="fb32")
        nc.sync.dma_start(fb32_sb[:], ins["fb32"][:])
        statsall = const.tile([KK, CH * D], BF16, tag="statsall")
        nc.gpsimd.memset(statsall[0:KS, :], 0.0)
        strips = []
        for s in range(NSTRIP):
            st = const.tile([KK, TKC], BF16, tag=f"strip{s}")
            nc.gpsimd.dma_start(st[KB:KK, :], ins["selpad"][:])
            strips.append(st)

        ident_sb = wsall_sb[:, 0:D]
        ws_ap = [wsall_sb[:, D * (1 + l):D * (2 + l)] for l in range(L)]
        statc_ap = smallbf_sb[0:KB, 0:D]
        wmean_ap = smallbf_sb[0:2, D:2 * D]

        # strip for chunk 0 ahead of the chain
        nc.sync.dma_start(strips[0][0:KB, :], ins["master"][:, 0:TKC])

        # ---------------- per-graph g3 chain (bf16, one pass) ----------
        # borrows psum from the main pool (slices of [128,1024] f32 tiles)
        # so all 8 banks stay available to the main loop; transposes run in
        # f32 against a cast identity. All 256 graphs in one F=256 pass.
        # Emission order matters (engines execute in program order): the
        # init phases of chunks 0..1 are emitted BEFORE the chain's
        # PE/ACT/DVE ops so the store stream starts while the chain runs;
        # the reduces are DVE's first ops, the locbar copies run on ACT.
        ps = ctx.enter_context(tc.tile_pool(name="ps", bufs=3, space="PSUM"))
        cps = ctx.enter_context(tc.tile_pool(name="cps", bufs=1, space="PSUM"))
        cbf = ctx.enter_context(tc.tile_pool(name="cbf", bufs=1, space="PSUM"))
        gtmp = ctx.enter_context(tc.tile_pool(name="gtmp", bufs=1))
        sFpool = ctx.enter_context(tc.tile_pool(name="sF", bufs=2))
        sIpool = ctx.enter_context(tc.tile_pool(name="sI", bufs=2))
        lowp = nc.allow_low_precision(reason="bf16 g3 chain vs 2e-2 gate")
        lowp.__enter__()
        lb = gtmp.tile([128, 4], BF16, tag="lb")
        lgk = lg[:].rearrange("p (g n k) -> p g k n", g=2, k=2)
        for g in range(2):
            for k in range(2):
                nc.vector.tensor_reduce(
                    lb[:, 2 * g + k:2 * g + k + 1], lgk[:, g, k:k + 1, :],
                    axis=mybir.AxisListType.X, op=mybir.AluOpType.add)

        def tiles(c):
            sFt = sFpool.tile([128, TKC], BF16, tag="sF", name=f"sF{c}")
            sIt = sIpool.tile([128, TKC], BF16, tag="sI", name=f"sI{c}")
            return sFt, sIt

        def phase_I(c, sIt):
            emit_phase(nc, ins, strips, statc_ap, KB, ps, sIt, out_init,
                       c, EVAC_PAT[c % 2][0:NR], prefetch=True)

        def phase_F(c, sFt):
            emit_phase(nc, ins, strips, statsall[:, D * c:D * (c + 1)], KK,
                       ps, sFt, out_final, c, EVAC_PAT[c % 2][NR:2 * NR],
                       prefetch=False)

        t01 = [tiles(0), tiles(1)]
        for c in (0, 1):
            phase_I(c, t01[c][1])

        # chain compute (PE/ACT/DVE ops land after the chunk-0/1 init
        # phases in each engine's program order)
        locbarT = gtmp.tile([2, BG], BF16, tag="locbarT")
        for g in range(2):
            tp = cbf.tile([KK, D], BF16, tag="tp", name=f"tp{g}")
            nc.tensor.transpose(tp[0:2, :], lb[:, 2 * g:2 * g + 2],
                                ident_sb)
            nc.scalar.activation(locbarT[:, 128 * g:128 * (g + 1)],
                                 tp[0:2, :], AF.Copy)

        mp = cps.tile([D, 2 * BG], F32, tag="cps")
        nc.tensor.matmul(mp[:, 0:BG], wmean_ap, locbarT[:],
                         start=True, stop=True)
        g_prev = gtmp.tile([128, BG], BF16, tag="g0")
        nc.scalar.activation(g_prev[:], mp[:, 0:BG], AF.Identity,
                             bias=fb32_sb[:, 0:1])
        for l in range(L):
            pp = cps.tile([D, 2 * BG], F32, tag="cps", name=f"pp{l}")
            nc.tensor.matmul(pp[:, 0:BG], ws_ap[l], g_prev[:],
                             start=True, stop=True)
            if l < L - 1:
                g_next = gtmp.tile([128, BG], BF16, tag=f"g{l + 1}")
                nc.scalar.activation(g_next[:], pp[:, 0:BG], AF.Relu,
                                     bias=fb32_sb[:, 1 + l:2 + l])
            else:
                g_next = gtmp.tile([128, BG], BF16, tag=f"g{l + 1}")
                nc.scalar.activation(g_next[:], pp[:, 0:BG], AF.Identity,
                                     bias=fb32_sb[:, 1 + l:2 + l])
            g_prev = g_next
        # per-chunk g3 stationary blocks: [32, 128] transposes into psum
        # partitions 32..63, then partition-preserving DVE copies
        for c in range(CH):
            tqc = cbf.tile([KK, D], BF16, tag="tp", name=f"tqc{c}")
            nc.tensor.transpose(
                tqc[KS:KK, :], g_prev[:, 32 * c:32 * (c + 1)], ident_sb)
            nc.vector.tensor_copy(
                statsall[KS:KK, D * c:D * (c + 1)], tqc[KS:KK, :])
        for c in range(CH):
            nc.vector.tensor_copy(statsall[0:KB, D * c:D * (c + 1)], statc_ap)
        lowp.__exit__(None, None, None)

        for c in (0, 1):
            phase_F(c, t01[c][0])

        def chunks(lo, hi):
            for c in range(lo, hi):
                sFt, sIt = tiles(c)
                phase_I(c, sIt)
                phase_F(c, sFt)

        if reps > 1:
            with tc.For_i(0, reps, 1):
                chunks(0, CH)
        else:
            chunks(2, CH)

    if split:
        _split_multiwaits(nc)
    return nc


def _evac(nc, eng, dst, src):
    """One 800-col PSUM f32 -> SBUF bf16 cast on the given engine.
    src: [128, 1024] psum tile (2 banks, MT used cols each);
    dst: [128, RT] slice of a store tile."""
    s3 = src.rearrange("p (b c) -> p b c", b=2)[:, :, 0:MT]
    d3 = dst.rearrange("p (b c) -> p b c", b=2)
    if eng == "V":
        nc.vector.tensor_copy(d3, s3)
    else:
        nc.scalar.activation(d3, s3, AF.Copy)


def emit_phase(nc, ins, strips, stat, rows, ps, sdt, out, c, pat,
               prefetch):
    st = strips[c % NSTRIP]
    if prefetch:
        # prefetch next chunk's strip (SP ring, ahead of this chunk's
        # stores)
        cn = (c + 1) % CH
        nc.sync.dma_start(strips[cn % NSTRIP][0:KB, :],
                          ins["master"][:, TKC * cn:TKC * (cn + 1)])
    for r in range(NR):
        tO = ps.tile([128, 2 * 512], F32, tag="ps", name=f"mm{c}r{r}")
        for q in range(2):
            nc.tensor.matmul(
                tO[:, 512 * q:512 * q + MT],
                stat,
                st[0:rows, RT * r + MT * q:RT * r + MT * (q + 1)],
                start=True, stop=True)
        _evac(nc, pat[r], sdt[:, RT * r:RT * (r + 1)], tO[:])
        if r % 2 == 1:
            # store each half as soon as its evacs land: keeps the DMA
            # engines saturated, shortens lead-in/drain
            hw = slice(TKC * c + RT * (r - 1), TKC * c + RT * (r + 1))
            nc.sync.dma_start(out[:, hw], sdt[:, RT * (r - 1):RT * (r + 1)])


def _bf_split(x, n=2):
    import ml_dtypes
    outs = []
    r = np.asarray(x, dtype=np.float32)
    for _ in range(n):
        h = r.astype(ml_dtypes.bfloat16)
        outs.append(h)
        r = r - h.astype(np.float32)
    return outs


def _prep_core_inputs(locs, W_init, b_init, Ws, bs):
    """Host-side shard + constant prep. Returns list of per-core input maps."""
    import ml_dtypes
    bfdt = ml_dtypes.bfloat16
    locs = np.ascontiguousarray(locs, dtype=np.float32)
    W_init = np.asarray(W_init, dtype=np.float32)
    b_init = np.asarray(b_init, dtype=np.float32)
    Ws = np.ascontiguousarray(Ws, dtype=np.float32)
    bs = np.asarray(bs, dtype=np.float32)

    # sel[j, u] = 1 iff chunk-local token u belongs to chunk-graph j;
    # preceded by 24 zero rows (strip partitions 8..31 pad)
    u = np.arange(TKC)
    sel = (u[None, :] // N == np.arange(GPC)[:, None]).astype(bfdt)
    selpad = np.ascontiguousarray(np.concatenate(
        [np.zeros((KS - KB, TKC), dtype=bfdt), sel], axis=0))

    Wh, Wl = _bf_split(W_init)
    bh, bl = _bf_split(b_init)
    wmean = (W_init / np.float32(N)).astype(bfdt)
    statc = np.stack([Wh[0], Wh[1], Wl[0], Wl[1], Wh[0], Wh[1], bh, bl])
    right = np.zeros((KB, D), dtype=np.float32)
    right[0:2] = wmean
    smallbf = np.ascontiguousarray(
        np.concatenate([statc, right], axis=1).astype(bfdt))
    fb32 = np.ascontiguousarray(np.concatenate(
        [b_init.reshape(D, 1), bs.T], axis=1).astype(np.float32))
    wsall = np.ascontiguousarray(np.concatenate(
        [np.eye(D, dtype=bfdt)] + [Ws[l].astype(bfdt) for l in range(L)],
        axis=1))

    in_maps = []
    for k in range(NCORES):
        lc = locs[BG * k:BG * (k + 1)]          # [256, 100, 2]
        lx, ly = lc[:, :, 0].ravel(), lc[:, :, 1].ravel()
        lxh, lxl = _bf_split(lx)
        lyh, lyl = _bf_split(ly)
        ones = np.ones(T, dtype=bfdt)
        master = np.stack([lxh, lyh, lxh, lyh, lxl, lyl, ones, ones])
        lc2 = lc.reshape(BG, 2 * N)
        locs2 = np.concatenate([lc2[:D], lc2[D:]], axis=1)
        in_maps.append({
            "master": np.ascontiguousarray(master.astype(bfdt)),
            "selpad": selpad,
            "smallbf": smallbf,
            "fb32": fb32,
            "wsall": wsall,
            "locs2": np.ascontiguousarray(locs2),
        })
    return in_maps


_CACHED_NC = None


def _get_nc():
    global _CACHED_NC
    if _CACHED_NC is None:
        _CACHED_NC = _build_program()
    return _CACHED_NC


def kernel(locs, W_init, b_init, Ws, bs, _trace=False):
    nc = _get_nc()
    in_maps = _prep_core_inputs(locs, W_init, b_init, Ws, bs)
    res = run_bass_kernel_spmd(nc, in_maps, list(range(NCORES)), trace=_trace)
    h = np.concatenate(
        [np.asarray(res.results[k]["out_final"]).astype(np.float32)
         .T.reshape(BG, N, D) for k in range(NCORES)], axis=0)
    init_h = np.concatenate(
        [np.asarray(res.results[k]["out_init"]).astype(np.float32)
         .T.reshape(BG, N, D) for k in range(NCORES)], axis=0)
    if _trace:
        return (h, init_h), res
    return (h, init_h)



# revision 2
# speedup vs baseline: 32.5696x; 32.5696x over previous
"""GCN encoder (nn_GCNEncoder) Trainium2 Bass kernel — bf16 feature-major.

Math: with a fully-connected graph + self loops, gcn_norm gives the uniform
adjacency A = 1/N. Then A @ X broadcasts mean_n(X) to every node, so after
layer 1 the node features are constant within each graph and the whole GCN
collapses to a per-graph vector chain:

  locbar[b] = mean_n locs[b, n, :]                       (R^2)
  g0[b]     = locbar[b] @ W_init + b_init                (R^D)
  g1        = relu(g0 @ Ws[0] + bs[0]); g2 = relu(g1 @ Ws[1] + bs[1])
  g3        = g2 @ Ws[2] + bs[2]
  init_h[b, n, :]  = locs[b, n, :] @ W_init + b_init
  h_final[b, n, :] = init_h[b, n, :] + g3[b, :]

Outputs (h_final, init_h) are 2 x [2048, 100, 128]; the kernel is purely
store-bandwidth bound. Both outputs are stored as bf16 (measured rel err
3.9e-3 vs the 2e-2 gate) in FEATURE-MAJOR layout [D, T] per core: every
partition writes 6.4 KB contiguous DRAM lines, keeping the DMA engines at
full rate with 2-byte elements; the host transposes after the gather.
Store traffic: 13.1 MB/core (~37 us at the modeled ~330 GB/s) vs ~73 us
for f32 — the baseline this replaces ran ~80-95 us.

Per-core schedule (256 graphs = 25600 tokens, 8 chunks of 32 graphs):
 - moving strip [64, 3200] bf16 per chunk (3 rotating buffers, prefetched
   a chunk ahead on the SP ring): rows 0..7 = [xh yh xh yh xl yl 1 1] per
   token (locs split bf16 hi+lo), rows 8..31 zero pad (matmul/transpose
   outputs must start at psum partition 0/32/64), rows 32..63 = graph
   one-hot (sel), pad+sel loaded once per buffer.
 - stationaries for all 8 chunks in one persistent [64, 8*128] tile:
   rows 0..7 = [W0h W1h W0l W1l W0h W1h bh bl] (bf16 products exact in
   f32 PSUM; dropped xl*Wl term ~2^-18 keeps init_h f32-exact before the
   store rounding), rows 8..31 zeros, rows 32..63 = bf16 g3 rows written
   by per-chunk PE transposes straight out of the chain (no staging
   DMAs). The init output uses rows 0..7 only.
 - per chunk: 8 init matmuls then 8 final matmuls of 400 cols (PE ~21
   us/core total), psum pool 3 x [128, 1024] f32; 800-col PSUM->SBUF
   bf16 evacuations alternate DVE/ACT (GPSIMD cannot access PSUM);
   half-chunk stores issue as soon as their evacs land.
 - g3 chain: one F=256 pass in bf16 (wmean matmul + 3 layers), dedicated
   1-bank psum pools, locbar via DVE reduces + PE transposes. Emission
   order exploits per-engine program order: chunk 0/1 INIT phases are
   emitted before the chain's PE/ACT/DVE ops, so the store stream starts
   ~7 us in while the chain computes; rings: SP = consts/strips/stores,
   Pool(gpsimd) = statsall memset + sel loads (serial ~1 us/DMA SWDGE).
 - measured on trn2 (8 cores): ~54-56 us/iteration steady-state
   (rep-delta), predicted ~52 us single-shot; rel err 3.9e-3.
"""

import numpy as np
from contextlib import ExitStack

import concourse.bass as bass
import concourse.mybir as mybir
import concourse.tile as tile
from concourse.bass_utils import run_bass_kernel_spmd

F32 = mybir.dt.float32
BF16 = mybir.dt.bfloat16
AF = mybir.ActivationFunctionType

B, N, D, L = 2048, 100, 128, 3
NCORES = 8
BG = B // NCORES          # 256 graphs per core
T = BG * N                # 25600 tokens per core
CH = 8                    # chunks per core
GPC = BG // CH            # 32 graphs per chunk
TKC = GPC * N             # 3200 tokens per chunk
KB = 8                    # base K rows (locs hi/lo pairs + ones)
KS = 32                   # sel/g3 block base partition (0/32/64 rule)
KK = KS + GPC             # 64 contraction rows (8..31 are zero pad)
MT = 400                  # matmul moving-tile columns
RT = 2 * MT               # columns per psum tile (2 banks)
NR = TKC // RT            # 4 rounds per chunk per output
NSTRIP = 3                # rotating strip buffers

# evacuation engine per (chunk parity, round, output): 9 ACT / 7 DVE
# per two chunks, matching the 1.2 : 0.96 GHz engine rates.
EVAC_PAT = [["A", "V", "A", "V", "A", "V", "A", "A"],
            ["V", "A", "V", "A", "V", "A", "V", "A"]]


def _split_multiwaits(nc, max_waits=1):
    """The walrus build in this container rejects instructions carrying more
    than one sync-wait command. Split extras into single-wait NoOps inserted
    immediately before the instruction (same engine, so sequencer order
    preserves semantics exactly)."""
    cnt = 0
    for f in nc.m.functions:
        for b in f.blocks:
            il = b.instructions
            i = 0
            while i < len(il):
                ins = il[i]
                si = ins.sync_info
                if si is not None and si.on_wait and len(si.on_wait) > max_waits:
                    waits = list(si.on_wait)
                    for w in waits[:-max_waits]:
                        nop = mybir.InstNoOp(name=f"I-SWAIT-{cnt}", ins=[], outs=[])
                        cnt += 1
                        nop.engine = ins.engine
                        nop.sync_info = mybir.SyncInfo(on_wait=[w], on_update=[])
                        il.insert(i, nop)
                        i += 1
                    ins.sync_info = mybir.SyncInfo(
                        on_wait=waits[-max_waits:],
                        on_update=list(si.on_update or []))
                i += 1
    return cnt


def _build_program(split=True, reps=1, timing=False):
    nc = bass.Bass("TRN2", target_bir_lowering=False, debug=False,
                   num_devices=NCORES)

    ins = {}
    for name, shape, dt in [
        ("master", [KB, T], BF16),         # coord rows + ones
        ("selpad", [KK - KB, TKC], BF16),  # 24 zero rows | 32 sel rows
        ("smallbf", [KB, 2 * D], BF16),    # cols 0:128 statc, 128:256 wmean (rows 0..1)
        ("fb32", [D, 4], F32),             # col 0 b_init, 1..3 bs.T
        ("wsall", [D, 4 * D], BF16),       # ident | Ws0 | Ws1 | Ws2
        ("locs2", [D, 4 * N], F32),        # graph p | graph p+128
    ]:
        ins[name] = nc.dram_tensor(name, shape, dt, kind="ExternalInput").ap()

    # timing builds keep the stores but land them in Internal DRAM so the
    # axon tunnel doesn't fetch 13 MB/core per timed call
    okind = "Internal" if timing else "ExternalOutput"
    out_final = nc.dram_tensor("out_final", [D, T], BF16, kind=okind).ap()
    out_init = nc.dram_tensor("out_init", [D, T], BF16, kind=okind).ap()
    if timing:
        nc.dram_tensor("tiny_out", [1, 4], mybir.dt.int32, kind="ExternalOutput")

    with tile.TileContext(nc) as tc, ExitStack() as ctx:
        const = ctx.enter_context(tc.tile_pool(name="const", bufs=1))

        # chain locs + chunk-0 strip on the SP ring first (critical
        # path); packed consts + sel on the Pool ring (a serial ~1 us/DMA
        # software-DGE path, ordered by first use) so the ACT/DVE
        # sequencers never stall behind DMA issues.
        lg = const.tile([D, 4 * N], F32, tag="lg")
        nc.sync.dma_start(lg[:], ins["locs2"][:])

        smallbf_sb = const.tile([KB, 2 * D], BF16, tag="smallbf")
        nc.sync.dma_start(smallbf_sb[:], ins["smallbf"][:])
        wsall_sb = const.tile([D, 4 * D], BF16, tag="wsall")
        nc.sync.dma_start(wsall_sb[:], ins["wsall"][:])
        fb32_sb = const.tile([D, 4], F32, tag="fb32")
        nc.sync.dma_start(fb32_sb[:], ins["fb32"][:])
        statsall = const.tile([KK, CH * D], BF16, tag="statsall")
        nc.gpsimd.memset(statsall[0:KS, :], 0.0)
        strips = []
        for s in range(NSTRIP):
            st = const.tile([KK, TKC], BF16, tag=f"strip{s}")
            nc.gpsimd.dma_start(st[KB:KK, :], ins["selpad"][:])
            strips.append(st)

        ident_sb = wsall_sb[:, 0:D]
        ws_ap = [wsall_sb[:, D * (1 + l):D * (2 + l)] for l in range(L)]
        statc_ap = smallbf_sb[0:KB, 0:D]
        wmean_ap = smallbf_sb[0:2, D:2 * D]

        # strip for chunk 0 ahead of the chain
        nc.sync.dma_start(strips[0][0:KB, :], ins["master"][:, 0:TKC])

        # ---------------- per-graph g3 chain (bf16, one pass) ----------
        # borrows psum from the main pool (slices of [128,1024] f32 tiles)
        # so all 8 banks stay available to the main loop; transposes run in
        # f32 against a cast identity. All 256 graphs in one F=256 pass.
        # Emission order matters (engines execute in program order): the
        # init phases of chunks 0..1 are emitted BEFORE the chain's
        # PE/ACT/DVE ops so the store stream starts while the chain runs;
        # the reduces are DVE's first ops, the locbar copies run on ACT.
        ps = ctx.enter_context(tc.tile_pool(name="ps", bufs=3, space="PSUM"))
        cps = ctx.enter_context(tc.tile_pool(name="cps", bufs=1, space="PSUM"))
        cbf = ctx.enter_context(tc.tile_pool(name="cbf", bufs=1, space="PSUM"))
        gtmp = ctx.enter_context(tc.tile_pool(name="gtmp", bufs=1))
        sFpool = ctx.enter_context(tc.tile_pool(name="sF", bufs=2))
        sIpool = ctx.enter_context(tc.tile_pool(name="sI", bufs=2))
        lowp = nc.allow_low_precision(reason="bf16 g3 chain vs 2e-2 gate")
        lowp.__enter__()
        lb = gtmp.tile([128, 4], BF16, tag="lb")
        lgk = lg[:].rearrange("p (g n k) -> p g k n", g=2, k=2)
        for g in range(2):
            for k in range(2):
                nc.vector.tensor_reduce(
                    lb[:, 2 * g + k:2 * g + k + 1], lgk[:, g, k:k + 1, :],
                    axis=mybir.AxisListType.X, op=mybir.AluOpType.add)

        def tiles(c):
            sFt = sFpool.tile([128, TKC], BF16, tag="sF", name=f"sF{c}")
            sIt = sIpool.tile([128, TKC], BF16, tag="sI", name=f"sI{c}")
            return sFt, sIt

        def phase_I(c, sIt):
            emit_phase(nc, ins, strips, statc_ap, KB, ps, sIt, out_init,
                       c, EVAC_PAT[c % 2][0:NR], prefetch=True)

        def phase_F(c, sFt):
            emit_phase(nc, ins, strips, statsall[:, D * c:D * (c + 1)], KK,
                       ps, sFt, out_final, c, EVAC_PAT[c % 2][NR:2 * NR],
                       prefetch=False)

        t01 = [tiles(0), tiles(1)]
        for c in (0, 1):
            phase_I(c, t01[c][1])

        # chain compute (PE/ACT/DVE ops land after the chunk-0/1 init
        # phases in each engine's program order)
        locbarT = gtmp.tile([2, BG], BF16, tag="locbarT")
        for g in range(2):
            tp = cbf.tile([KK, D], BF16, tag="tp", name=f"tp{g}")
            nc.tensor.transpose(tp[0:2, :], lb[:, 2 * g:2 * g + 2],
                                ident_sb)
            nc.scalar.activation(locbarT[:, 128 * g:128 * (g + 1)],
                                 tp[0:2, :], AF.Copy)

        mp = cps.tile([D, 2 * BG], F32, tag="cps")
        nc.tensor.matmul(mp[:, 0:BG], wmean_ap, locbarT[:],
                         start=True, stop=True)
        g_prev = gtmp.tile([128, BG], BF16, tag="g0")
        nc.scalar.activation(g_prev[:], mp[:, 0:BG], AF.Identity,
                             bias=fb32_sb[:, 0:1])
        for l in range(L):
            pp = cps.tile([D, 2 * BG], F32, tag="cps", name=f"pp{l}")
            nc.tensor.matmul(pp[:, 0:BG], ws_ap[l], g_prev[:],
                             start=True, stop=True)
            if l < L - 1:
                g_next = gtmp.tile([128, BG], BF16, tag=f"g{l + 1}")
                nc.scalar.activation(g_next[:], pp[:, 0:BG], AF.Relu,
                                     bias=fb32_sb[:, 1 + l:2 + l])
            else:
                g_next = gtmp.tile([128, BG], BF16, tag=f"g{l + 1}")
                nc.scalar.activation(g_next[:], pp[:, 0:BG], AF.Identity,
                                     bias=fb32_sb[:, 1 + l:2 + l])
            g_prev = g_next
        # per-chunk g3 stationary blocks: [32, 128] transposes into psum
        # partitions 32..63, then partition-preserving DVE copies
        for c in range(CH):
            tqc = cbf.tile([KK, D], BF16, tag="tp", name=f"tqc{c}")
            nc.tensor.transpose(
                tqc[KS:KK, :], g_prev[:, 32 * c:32 * (c + 1)], ident_sb)
            nc.vector.tensor_copy(
                statsall[KS:KK, D * c:D * (c + 1)], tqc[KS:KK, :])
        for c in range(CH):
            nc.vector.tensor_copy(statsall[0:KB, D * c:D * (c + 1)], statc_ap)
        lowp.__exit__(None, None, None)

        for c in (0, 1):
            phase_F(c, t01[c][0])

        def chunks(lo, hi):
            for c in range(lo, hi):
                sFt, sIt = tiles(c)
                phase_I(c, sIt)
                phase_F(c, sFt)

        if reps > 1:
            with tc.For_i(0, reps, 1):
                chunks(0, CH)
        else:
            chunks(2, CH)

    if split:
        _split_multiwaits(nc)
    return nc


def _evac(nc, eng, dst, src):
    """One 800-col PSUM f32 -> SBUF bf16 cast on the given engine.
    src: [128, 1024] psum tile (2 banks, MT used cols each);
    dst: [128, RT] slice of a store tile."""
    s3 = src.rearrange("p (b c) -> p b c", b=2)[:, :, 0:MT]
    d3 = dst.rearrange("p (b c) -> p b c", b=2)
    if eng == "V":
        nc.vector.tensor_copy(d3, s3)
    else:
        nc.scalar.activation(d3, s3, AF.Copy)


def emit_phase(nc, ins, strips, stat, rows, ps, sdt, out, c, pat,
               prefetch):
    st = strips[c % NSTRIP]
    if prefetch:
        # prefetch next chunk's strip (SP ring, ahead of this chunk's
        # stores)
        cn = (c + 1) % CH
        nc.sync.dma_start(strips[cn % NSTRIP][0:KB, :],
                          ins["master"][:, TKC * cn:TKC * (cn + 1)])
    for r in range(NR):
        tO = ps.tile([128, 2 * 512], F32, tag="ps", name=f"mm{c}r{r}")
        for q in range(2):
            nc.tensor.matmul(
                tO[:, 512 * q:512 * q + MT],
                stat,
                st[0:rows, RT * r + MT * q:RT * r + MT * (q + 1)],
                start=True, stop=True)
        _evac(nc, pat[r], sdt[:, RT * r:RT * (r + 1)], tO[:])
        if r % 2 == 1:
            # store each half as soon as its evacs land: keeps the DMA
            # engines saturated, shortens lead-in/drain
            hw = slice(TKC * c + RT * (r - 1), TKC * c + RT * (r + 1))
            nc.sync.dma_start(out[:, hw], sdt[:, RT * (r - 1):RT * (r + 1)])


def _bf_split(x, n=2):
    import ml_dtypes
    outs = []
    r = np.asarray(x, dtype=np.float32)
    for _ in range(n):
        h = r.astype(ml_dtypes.bfloat16)
        outs.append(h)
        r = r - h.astype(np.float32)
    return outs


def _prep_core_inputs(locs, W_init, b_init, Ws, bs):
    """Host-side shard + constant prep. Returns list of per-core input maps."""
    import ml_dtypes
    bfdt = ml_dtypes.bfloat16
    locs = np.ascontiguousarray(locs, dtype=np.float32)
    W_init = np.asarray(W_init, dtype=np.float32)
    b_init = np.asarray(b_init, dtype=np.float32)
    Ws = np.ascontiguousarray(Ws, dtype=np.float32)
    bs = np.asarray(bs, dtype=np.float32)

    # sel[j, u] = 1 iff chunk-local token u belongs to chunk-graph j;
    # preceded by 24 zero rows (strip partitions 8..31 pad)
    u = np.arange(TKC)
    sel = (u[None, :] // N == np.arange(GPC)[:, None]).astype(bfdt)
    selpad = np.ascontiguousarray(np.concatenate(
        [np.zeros((KS - KB, TKC), dtype=bfdt), sel], axis=0))

    Wh, Wl = _bf_split(W_init)
    bh, bl = _bf_split(b_init)
    wmean = (W_init / np.float32(N)).astype(bfdt)
    statc = np.stack([Wh[0], Wh[1], Wl[0], Wl[1], Wh[0], Wh[1], bh, bl])
    right = np.zeros((KB, D), dtype=np.float32)
    right[0:2] = wmean
    smallbf = np.ascontiguousarray(
        np.concatenate([statc, right], axis=1).astype(bfdt))
    fb32 = np.ascontiguousarray(np.concatenate(
        [b_init.reshape(D, 1), bs.T], axis=1).astype(np.float32))
    wsall = np.ascontiguousarray(np.concatenate(
        [np.eye(D, dtype=bfdt)] + [Ws[l].astype(bfdt) for l in range(L)],
        axis=1))

    in_maps = []
    for k in range(NCORES):
        lc = locs[BG * k:BG * (k + 1)]          # [256, 100, 2]
        lx, ly = lc[:, :, 0].ravel(), lc[:, :, 1].ravel()
        lxh, lxl = _bf_split(lx)
        lyh, lyl = _bf_split(ly)
        ones = np.ones(T, dtype=bfdt)
        master = np.stack([lxh, lyh, lxh, lyh, lxl, lyl, ones, ones])
        lc2 = lc.reshape(BG, 2 * N)
        locs2 = np.concatenate([lc2[:D], lc2[D:]], axis=1)
        in_maps.append({
            "master": np.ascontiguousarray(master.astype(bfdt)),
            "selpad": selpad,
            "smallbf": smallbf,
            "fb32": fb32,
            "wsall": wsall,
            "locs2": np.ascontiguousarray(locs2),
        })
    return in_maps


_CACHED_NC = None


def _get_nc():
    global _CACHED_NC
    if _CACHED_NC is None:
        _CACHED_NC = _build_program()
    return _CACHED_NC


def kernel(locs, W_init, b_init, Ws, bs, _trace=False):
    nc = _get_nc()
    in_maps = _prep_core_inputs(locs, W_init, b_init, Ws, bs)
    res = run_bass_kernel_spmd(nc, in_maps, list(range(NCORES)), trace=_trace)
    h = np.concatenate(
        [np.asarray(res.results[k]["out_final"]).astype(np.float32)
         .T.reshape(BG, N, D) for k in range(NCORES)], axis=0)
    init_h = np.concatenate(
        [np.asarray(res.results[k]["out_init"]).astype(np.float32)
         .T.reshape(BG, N, D) for k in range(NCORES)], axis=0)
    if _trace:
        return (h, init_h), res
    return (h, init_h)

